# revision 37
# baseline (speedup 1.0000x reference)
"""DifferentialAttention Trainium2 kernel (8-core SPMD), v2.

Sharding: 8 cores = 4 batches x 2 head-groups (8 heads each).

v2 structure (vs v1):
  - all inputs bf16 (host-cast): halves DMA + SBUF, same PE rate.
  - DMA lands directly in matmul-ready tiles (no cast copies).
  - per-head Q/K projection pipelined with the previous head's
    attention, so exp (ACT) overlaps projection matmuls (PE).
  - scores/u matmuls narrowed to the causal triangle (bf16 runs full
    rate at any width).
  - s1/s2 paired in one PSUM tile -> single exp + single affine_select.
  - u1/u2 share one PSUM tile; one reciprocal after the broadcast;
    -lambda folded into a second V copy (vau2 = [-lam*V | 1]).
  - GroupNorm folded into the output projection: Wc rows pre-scaled by
    rstd per head, mean handled via a broadcast bias correction, so y
    needs no normalize pass and the tail is short.
Host sums the two partials per batch (the "all-reduce after").
"""

import math
import sys

for _p in ("/opt/trn_rl_repo", "/root/.axon_site/_ro/trn_rl_repo"):
    if _p not in sys.path:
        sys.path.append(_p)

from contextlib import ExitStack

import numpy as np

import concourse.mybir as mybir
import concourse.tile as tile
from concourse import bacc
from concourse.bass_utils import run_bass_kernel_spmd

F32 = mybir.dt.float32
F32R = mybir.dt.float32r
BF16 = mybir.dt.bfloat16
AF = mybir.ActivationFunctionType
OP = mybir.AluOpType

B, T, C = 4, 1024, 1024
NH = 16
HD = C // NH  # 64
NHL = 8  # heads per core
LAMBDA_INIT = 0.8 - 0.6 * math.exp(-0.3 * 1.0)
EPS = 1e-5
SCALE = 1.0 / math.sqrt(HD)
N_CORES = 8
NKT = T // 128  # 8 token tiles
NKC = C // 128  # 8 contraction tiles


def _bcast(dram_tile, parts, width):
    import concourse.bass as bass

    ap = dram_tile[:]
    return bass.AP(tensor=ap.tensor, offset=ap.offset, ap=[[0, parts], [1, width]])


def build_program(n_iters: int = 1):
    nc = bacc.Bacc("TRN2", target_bir_lowering=False, debug=False)
    x_d = nc.dram_tensor("xbT", [C, T], BF16, kind="ExternalInput").ap()
    wq_d = nc.dram_tensor("wq", [C, 1024], BF16, kind="ExternalInput").ap()
    wk_d = nc.dram_tensor("wk", [C, 1024], BF16, kind="ExternalInput").ap()
    wv_d = nc.dram_tensor("wv", [C, 512], BF16, kind="ExternalInput").ap()
    wc_d = nc.dram_tensor("wc", [512, C], BF16, kind="ExternalInput").ap()
    neglam_d = nc.dram_tensor("neglam", [128, 1], F32, kind="ExternalInput").ap()
    out_d = nc.dram_tensor("outp", [T, C], F32, kind="ExternalOutput").ap()

    with tile.TileContext(nc) as tc, ExitStack() as ctx:
        if n_iters == 1:
            _emit_iteration(nc, tc, x_d, wq_d, wk_d, wv_d, wc_d, neglam_d, out_d)
        else:
            with tc.For_i(0, n_iters, 1):
                _emit_iteration(nc, tc, x_d, wq_d, wk_d, wv_d, wc_d, neglam_d, out_d)

    nc.compile()
    return nc


def _emit_iteration(nc, tc, x_d, wq_d, wk_d, wv_d, wc_d, neglam_d, out_d):
    with ExitStack() as ctx:
        lp = ctx.enter_context(tc.tile_pool(name="long", bufs=1))
        qk = ctx.enter_context(tc.tile_pool(name="qk", bufs=2))
        sm = ctx.enter_context(tc.tile_pool(name="sm", bufs=2))
        psq = ctx.enter_context(tc.tile_pool(name="psq", bufs=2, space="PSUM"))
        pss = ctx.enter_context(tc.tile_pool(name="pss", bufs=2, space="PSUM"))
        psu = ctx.enter_context(tc.tile_pool(name="psu", bufs=1, space="PSUM"))
        drp = ctx.enter_context(tc.tile_pool(name="drp", bufs=4, space="DRAM"))

        # ---------------- consts ----------------
        neglamv = lp.tile([128, 1], F32, tag="neglamv")
        nc.sync.dma_start(neglamv[:], neglam_d)
        ones64 = lp.tile([64, 64], F32, tag="ones64")
        nc.vector.memset(ones64[:], 1.0)
        epsc = lp.tile([64, 1], F32, tag="epsc")
        nc.vector.memset(epsc[:], EPS)
        lnb = lp.tile([64, 1], F32, tag="lnb")
        nc.vector.memset(lnb[:], float(math.log(1.0 - LAMBDA_INIT)))

        # ---------------- input DMAs (land directly, no casts) ----------
        # order: wv first (V proj gates head 0), then x, then wq/wk
        # interleaved (Q and K chains finish about together), wc last.
        wv_t = [lp.tile([128, 512], BF16, tag=f"wv{k}", name=f"wv{k}") for k in range(NKC)]
        for k in range(NKC):
            nc.sync.dma_start(wv_t[k][:], wv_d[k * 128 : (k + 1) * 128, :])
        xT = [lp.tile([128, T], BF16, tag=f"xt{k}", name=f"xt{k}") for k in range(NKC)]
        for k in range(NKC):
            nc.sync.dma_start(xT[k][:], x_d[k * 128 : (k + 1) * 128, :])
        wq_t = [lp.tile([128, 1024], BF16, tag=f"wq{k}", name=f"wq{k}") for k in range(NKC)]
        wk_t = [lp.tile([128, 1024], BF16, tag=f"wk{k}", name=f"wk{k}") for k in range(NKC)]
        for k in range(NKC):
            nc.sync.dma_start(wq_t[k][:], wq_d[k * 128 : (k + 1) * 128, :])
            nc.sync.dma_start(wk_t[k][:], wk_d[k * 128 : (k + 1) * 128, :])
        wcs = [lp.tile([128, C], BF16, tag=f"wcs{k}", name=f"wcs{k}") for k in range(4)]
        for k in range(4):
            nc.sync.dma_start(wcs[k][:], wc_d[k * 128 : (k + 1) * 128, :])

        # ---------------- persistent SBUF state ----------------
        # vaug: [V | 1] per token-tile, vau2: [-lam*V | 1]
        vaug = [lp.tile([128, NHL, HD + 1], BF16, tag=f"vaug{t}", name=f"vaug{t}") for t in range(NKT)]
        vau2 = [lp.tile([128, NHL, HD + 1], BF16, tag=f"vau2{t}", name=f"vau2{t}") for t in range(NKT)]
        for t in range(NKT):
            nc.vector.memset(vaug[t][:, :, HD : HD + 1], 1.0)
            nc.gpsimd.memset(vau2[t][:, :, HD : HD + 1], 1.0)
        # p tiles: [128, 2, w] (p1,p2 paired), w = T - (i//4)*512
        p_t = [
            lp.tile([128, 2, T - (i // 4) * 512], BF16, tag=f"p{i}", name=f"p{i}") for i in range(NKT)
        ]
        # yTn: [128, T] bf16, rows 0:64 = head 2kk, 64:128 = head 2kk+1
        yTn = [lp.tile([128, T], BF16, tag=f"ytn{kk}", name=f"ytn{kk}") for kk in range(4)]
        # scaled output weights, bias vectors
        wcp = [lp.tile([128, C], BF16, tag=f"wcp{kk}", name=f"wcp{kk}") for kk in range(4)]
        mvecb = [lp.tile([128, 1], BF16, tag=f"mvb{kk}", name=f"mvb{kk}") for kk in range(4)]
        # per-partition stats for all heads: cols [mean_0..7 | (var+m^2)_0..7]
        stAll = lp.tile([64, 16], F32, tag="stAll")

        # ---------------- V projection ----------------
        def emit_v(t):
            pv = psq.tile([128, 512], F32, tag="pq", name=f"pv{t}")
            for k in range(NKC):
                nc.tensor.matmul(
                    pv[:],
                    xT[k][:, t * 128 : (t + 1) * 128],
                    wv_t[k][:],
                    start=(k == 0),
                    stop=(k == NKC - 1),
                )
            pvv = pv[:].rearrange("p (h d) -> p h d", h=NHL)
            nc.scalar.copy(vaug[t][:, :, 0:HD], pvv)
            nc.vector.tensor_scalar_mul(vau2[t][:, :, 0:HD], pvv, neglamv[:])

        # ---------------- Q/K projection for one head ----------------
        # emitted as 4 separate chain closures so they can be interleaved
        # between score chunks (PE fills exp-wait stalls with projection).
        def emit_qk_alloc(j):
            QT_j = qk.tile([128, T], BF16, tag="QT", name=f"QT{j}")
            KT_j = qk.tile([128, T], BF16, tag="KT", name=f"KT{j}")
            return QT_j, KT_j

        def emit_qk_chain(j, cur, part):
            QT_j, KT_j = cur
            name, wt, dst = ("q", wq_t, QT_j) if part < 2 else ("k", wk_t, KT_j)
            ch = part % 2
            pq = psq.tile([128, 512], F32, tag="pq", name=f"p{name}{j}_{ch}")
            for k in range(NKC):
                nc.tensor.matmul(
                    pq[:],
                    wt[k][:, j * 128 : (j + 1) * 128],
                    xT[k][:, ch * 512 : (ch + 1) * 512],
                    start=(k == 0),
                    stop=(k == NKC - 1),
                )
            dstap = dst[:, ch * 512 : (ch + 1) * 512]
            # all four on DVE: on ACT they head-of-line block the exps, on
            # Pool the affine_selects; DVE's downstream (division -> stats)
            # has slack.
            nc.vector.tensor_copy(dstap, pq[:])

        # ---------------- scores + exp + mask for one head ----------------
        # nxt: (j+1, alloc) for interleaved projection chain emission
        def emit_scores(j, QT_j, KT_j, nxt=None):
            for i in range(NKT):
                if nxt is not None and 1 <= i <= 4:
                    emit_qk_chain(nxt[0], nxt[1], i - 1)
                c0 = (i // 4) * 512  # p-tile col base
                d0 = i * 128 - c0  # diag offset within p-tile
                nch = (T - c0) // 512
                for cb in range(nch):
                    a = d0 if cb == 0 else cb * 512  # start col in p-tile coords
                    b = (cb + 1) * 512
                    s = pss.tile([128, 2, 512], F32, tag="s", name=f"s{j}_{i}_{cb}")
                    sa = a - cb * 512
                    nc.tensor.matmul(
                        s[:, 0, sa:512],
                        KT_j[0:64, i * 128 : (i + 1) * 128],
                        QT_j[0:64, c0 + a : c0 + b],
                        start=True,
                        stop=True,
                        tile_position=(0, 0),
                    )
                    nc.tensor.matmul(
                        s[:, 1, sa:512],
                        KT_j[64:128, i * 128 : (i + 1) * 128],
                        QT_j[64:128, c0 + a : c0 + b],
                        start=True,
                        stop=True,
                        tile_position=(64, 0),
                    )
                    nc.scalar.activation(
                        p_t[i][:, :, a:b], s[:, :, sa:512], AF.Exp, scale=SCALE
                    )
                for half in range(2):
                    nc.gpsimd.affine_select(
                        out=p_t[i][:, half, d0 : d0 + 128],
                        in_=p_t[i][:, half, d0 : d0 + 128],
                        compare_op=OP.is_ge,
                        fill=0.0,
                        base=0,
                        pattern=[[1, 128]],
                        channel_multiplier=-1,
                    )

        # ---------------- u matmuls + division for one (head, chunk) -----
        # returns the [64, 512] bf16 y tile (or yTn slice) holding the result
        def emit_u(j, c, kk, odd):
            ilast = min(NKT, (c + 1) * 4)
            u = psu.tile([65, 2, 512], F32, tag="u", name=f"u{j}_{c}")
            for half, va in ((0, vaug), (1, vau2)):
                for i in range(ilast):
                    c0 = (i // 4) * 512
                    d0 = i * 128 - c0
                    lo = c * 512 - c0  # chunk start in p-tile coords
                    a = max(lo, d0)
                    nc.tensor.matmul(
                        u[:, half, a - lo : 512],
                        va[i][:, j, :],
                        p_t[i][:, half, a : lo + 512],
                        start=(i == 0),
                        stop=(i == ilast - 1),
                        skip_group_check=True,
                    )
            usb = sm.tile([65, 2, 512], F32, tag="usb", name=f"usb{j}_{c}")
            nc.vector.tensor_copy(usb[:], u[:])
            db = drp.tile([1, 1024], F32, tag="db", name=f"db{j}_{c}")
            nc.sync.dma_start(db[:], usb[64:65, :, :])
            R = sm.tile([64, 2, 512], F32, tag="R", name=f"R{j}_{c}")
            nc.sync.dma_start(R[:], _bcast(db, 64, 1024))
            Rr = sm.tile([64, 2, 512], F32, tag="Rr", name=f"Rr{j}_{c}")
            nc.vector.reciprocal(Rr[:], R[:])
            r1, r2 = Rr[:, 0, :], Rr[:, 1, :]
            num = usb
            t1 = sm.tile([64, 512], F32, tag="t1", name=f"t1{j}_{c}")
            nc.vector.tensor_tensor(t1[:], num[0:64, 0, :], r1, OP.mult)
            t2 = sm.tile([64, 512], F32, tag="t2", name=f"t2{j}_{c}")
            nc.vector.tensor_tensor(t2[:], num[0:64, 1, :], r2, OP.mult)
            if not odd:
                ydst = yTn[kk][0:64, c * 512 : (c + 1) * 512]
                nc.gpsimd.tensor_tensor(ydst, t1[:], t2[:], OP.add)
                return ydst
            yt = sm.tile([64, 512], BF16, tag="yt", name=f"yt{j}_{c}")
            nc.gpsimd.tensor_tensor(yt[:], t1[:], t2[:], OP.add)
            nc.sync.dma_start(yTn[kk][64:128, c * 512 : (c + 1) * 512], yt[:])
            return yt[:]

        # ---------------- per-head stats (always on partitions 0:64) ----
        # only writes per-partition stats into stAll; the cross-partition
        # reduce happens once at the tail (no mid-kernel PSUM pressure).
        def emit_stats(j, kk, hp, ysrc):
            bstats = sm.tile([64, 2, 6], F32, tag="bst", name=f"bst{j}")
            for si in range(2):
                nc.vector.bn_stats(out=bstats[:, si, :], in_=ysrc[si])
            mv = sm.tile([64, 2], F32, tag="mv", name=f"mv{j}")
            nc.vector.bn_aggr(out=mv[:], in_=bstats[:])
            jj = (j % 2) * 4 + j // 2
            nc.vector.tensor_copy(stAll[:, jj : jj + 1], mv[:, 0:1])
            nc.vector.scalar_tensor_tensor(
                out=stAll[:, 8 + jj : 9 + jj],
                in0=mv[:, 0:1],
                scalar=mv[:, 0:1],
                in1=mv[:, 1:2],
                op0=OP.mult,
                op1=OP.add,
            )

        # ---------------- pair finalize, split in two passes -------------
        # pass A (during head 7): pairs 0..2 from a full-width reduce gated
        # after head-6 exps (cols of heads 6/7 are garbage, never read);
        # pass B (tail): pair 3 only. Ln/Exp batched per pass.
        rvecA = lp.tile([128, 4], F32, tag="rvecA")
        mvfA = lp.tile([128, 4], F32, tag="mvfA")

        def emit_pairs(phase):
            lo, hi, tag = (0, 3, "A") if phase == 0 else (3, 4, "B")
            pst = pss.tile([128, 2, 512], F32, tag="s", name=f"pst{tag}")
            nc.tensor.matmul(
                pst[0:64, 0, 0:16], ones64[:], stAll[:], start=True, stop=True
            )
            statsA = sm.tile([64, 16], F32, tag="statsA", name=f"stats{tag}")
            nc.vector.tensor_scalar_mul(statsA[:], pst[0:64, 0, 0:16], 1.0 / 64.0)
            n = hi - lo
            m2 = sm.tile([64, 8], F32, tag="m2", name=f"m2{tag}")
            varA = sm.tile([64, 8], F32, tag="varA", name=f"var{tag}")
            lnv = sm.tile([64, 8], F32, tag="lnv", name=f"lnv{tag}")
            rstd = sm.tile([64, 8], F32, tag="rstd", name=f"rstd{tag}")
            for half in range(2):
                a = 4 * half + lo
                me = statsA[:, a : a + n]
                nc.vector.tensor_tensor(m2[:, a : a + n], me, me, OP.mult)
                nc.vector.tensor_tensor(
                    varA[:, a : a + n], statsA[:, 8 + a : 8 + a + n], m2[:, a : a + n], OP.subtract
                )
                nc.scalar.activation(
                    lnv[:, a : a + n], varA[:, a : a + n], AF.Ln, bias=epsc[:]
                )
                # rstd = (1 - lambda_init) / sqrt(var + eps)
                nc.scalar.activation(
                    rstd[:, a : a + n], lnv[:, a : a + n], AF.Exp, scale=-0.5, bias=lnb[:]
                )
            nc.vector.tensor_copy(rvecA[0:64, lo:hi], rstd[:, lo:hi])
            nc.sync.dma_start(rvecA[64:128, lo:hi], rstd[:, 4 + lo : 4 + hi])
            nc.vector.tensor_copy(mvfA[0:64, lo:hi], statsA[:, lo:hi])
            nc.sync.dma_start(mvfA[64:128, lo:hi], statsA[:, 4 + lo : 4 + hi])
            for kk in range(lo, hi):
                nc.vector.tensor_scalar_mul(wcp[kk][:], wcs[kk][:], rvecA[:, kk : kk + 1])
                nc.gpsimd.tensor_copy(mvecb[kk][:], mvfA[:, kk : kk + 1])

        # ---------------- emit pipeline ----------------
        for t in range(4):
            emit_v(t)
        cur = emit_qk_alloc(0)
        for part in range(4):
            emit_qk_chain(0, cur, part)
        for j in range(NHL):
            kk, odd = j // 2, j % 2
            QT_j, KT_j = cur
            if j < NHL - 1:
                nxt = (j + 1, emit_qk_alloc(j + 1))
            else:
                nxt = None
            emit_scores(j, QT_j, KT_j, nxt)
            if j == 0:
                for t in range(4, NKT):
                    emit_v(t)
            if nxt is not None:
                cur = nxt[1]
            y0 = emit_u(j, 0, kk, odd)
            y1 = emit_u(j, 1, kk, odd)
            if odd:
                ysrc = (y0, y1)
            else:
                ysrc = (
                    yTn[kk][0:64, 0:512],
                    yTn[kk][0:64, 512:1024],
                )
            emit_stats(j, kk, odd, ysrc)
        emit_pairs(0)
        emit_pairs(1)

        # ---------------- negbias = -sum_kk mvec_kk^T @ wcp_kk ----------
        nbs = []
        for ch in range(2):
            nbp = psq.tile([128, 512], F32, tag="pq", name=f"nbp{ch}")
            for kk in range(4):
                nc.tensor.matmul(
                    nbp[0:1, :],
                    mvecb[kk][:],
                    wcp[kk][:, ch * 512 : (ch + 1) * 512],
                    start=(kk == 0),
                    stop=(kk == 3),
                )
            nbs.append(nbp)
        negbias = sm.tile([1, 2, 512], F32, tag="nb", name="negbias")
        for ch in range(2):
            nc.vector.tensor_scalar_mul(negbias[:, ch, :], nbs[ch][0:1, :], -1.0)
        # broadcast negbias across partitions via a DRAM roundtrip
        dbn = drp.tile([1, 1024], F32, tag="db", name="dbn")
        nc.sync.dma_start(dbn[:], negbias[0:1, :, :])
        NBt = lp.tile([128, 2, 512], F32, tag="NBt")
        nc.sync.dma_start(NBt[:], _bcast(dbn, 128, 1024))
        NB = [NBt[:, 0, :], NBt[:, 1, :]]

        # ---------------- output projection ----------------
        for m in range(NKC):
            po = pss.tile([128, 2, 512], F32, tag="s", name=f"po{m}")
            for ch in range(2):
                for kk in range(4):
                    nc.tensor.matmul(
                        po[:, ch, :],
                        yTn[kk][:, m * 128 : (m + 1) * 128],
                        wcp[kk][:, ch * 512 : (ch + 1) * 512],
                        start=(kk == 0),
                        stop=(kk == 3),
                    )
            osb = sm.tile([128, 2, 512], F32, tag="osb", name=f"osb{m}")
            for ch in range(2):
                nc.vector.tensor_tensor(osb[:, ch, :], po[:, ch, :], NB[ch], OP.add)
            nc.sync.dma_start(
                out_d[m * 128 : (m + 1) * 128, :],
                osb[:].rearrange("p c w -> p (c w)"),
            )


_PROGRAM_CACHE = {}


def get_program(n_iters: int = 1):
    if n_iters not in _PROGRAM_CACHE:
        _PROGRAM_CACHE[n_iters] = build_program(n_iters)
    return _PROGRAM_CACHE[n_iters]


def make_in_maps(x, Wq, Wk, Wv, Wc, lambda_q1, lambda_k1, lambda_q2, lambda_k2):
    import ml_dtypes

    bf16 = ml_dtypes.bfloat16
    lam = (
        math.exp(float(np.sum(lambda_q1.astype(np.float64) * lambda_k1.astype(np.float64))))
        - math.exp(float(np.sum(lambda_q2.astype(np.float64) * lambda_k2.astype(np.float64))))
        + LAMBDA_INIT
    )
    neglam = np.full((128, 1), -lam, dtype=np.float32)
    in_maps = []
    for core in range(N_CORES):
        b, g = core // 2, core % 2
        in_maps.append(
            {
                "xbT": np.ascontiguousarray(x[b].T).astype(bf16),
                "wq": np.ascontiguousarray(Wq[:, g * 1024 : (g + 1) * 1024]).astype(bf16),
                "wk": np.ascontiguousarray(Wk[:, g * 1024 : (g + 1) * 1024]).astype(bf16),
                "wv": np.ascontiguousarray(Wv[:, g * 512 : (g + 1) * 512]).astype(bf16),
                "wc": np.ascontiguousarray(Wc[g * 512 : (g + 1) * 512, :]).astype(bf16),
                "neglam": neglam,
            }
        )
    return in_maps


def kernel(x, Wq, Wk, Wv, Wc, lambda_q1, lambda_k1, lambda_q2, lambda_k2):
    x = np.asarray(x, dtype=np.float32)
    in_maps = make_in_maps(
        x,
        np.asarray(Wq, np.float32),
        np.asarray(Wk, np.float32),
        np.asarray(Wv, np.float32),
        np.asarray(Wc, np.float32),
        np.asarray(lambda_q1, np.float32),
        np.asarray(lambda_k1, np.float32),
        np.asarray(lambda_q2, np.float32),
        np.asarray(lambda_k2, np.float32),
    )
    nc = get_program(1)
    res = run_bass_kernel_spmd(nc, in_maps, list(range(N_CORES)))
    out = np.empty((B, T, C), dtype=np.float32)
    for b in range(B):
        out[b] = res.results[2 * b]["outp"] + res.results[2 * b + 1]["outp"]
    return out
